# revision 2
# baseline (speedup 1.0000x reference)
"""ConvSelfAttention Trainium2 kernel.

Reference computation (per batch b, with x flattened to [C=128, N=4096]):
    q = wq @ x + bq        [64, N]   (1/sqrt(128) folded into wq/bq)
    k = wk @ x + bk        [64, N]
    v = wv @ x + bv        [64, N]
    s[i,j] = sum_o q[o,i] k[o,j]
    p = softmax_j(s)
    out[o,i] = sum_j v[o,j] p[i,j]
    y = gamma * (wo @ out + bo) + x

Mapping (one batch per NeuronCore, 8 cores):
  - scores are built TRANSPOSED: sT[j,i] = sum_o k[o,j] q[o,i]; q/k are kept
    DUPLICATED in both partition halves so consecutive j-tiles run
    CONCURRENTLY in the PE array via row tile_position (0,0)/(64,0).
  - exp is SPLIT between two engines, per pair of j-tiles ([128,1024] psum):
      'S' pairs: ScalarE ACT Exp (fp32 -> bf16 pT).
      'D' pairs: DVE Schraudolph fast-exp in ONE tensor_scalar:
        t = s*(128/ln2) + (2^23 + 16256 - 5.605); the fp32 add rounds the
        low mantissa to an integer that IS the bf16 bit pattern of
        ~exp(s) (max rel err ~3%); the bf16 strided bitcast view
        (step 2, low halves) feeds the PV matmul directly.
  - PV keeps the ones-augmented V^T STATIONARY ([128 j, 65], col 64 = ones):
    psum U[o,i] accumulates over all 32 j-tiles in one bank; row 64 is the
    softmax denominator D[i]. One 65-col LDW per 512-col matmul (vs a full
    128x128 pT reload per 65-col matmul) keeps the weight port cold.
  - denominator: rden = Exp(-Ln(D)) on ScalarE (both funcs in the
    natural_log_exp table set -> single ACT_TABLE_LOAD), broadcast across
    partitions with a K=1 ones-matmul at partition 64, then one DVE
    tensor_tensor multiply normalizes U.
  - output projection contracts o=65: row 64 of the weight is
    gamma*(wo@bv+bo) against the constant ones row of Ub (sum_j p = 1), so
    bias and v-bias come for free; gamma folds into wo on the host.
"""

import sys

import numpy as np

try:
    import concourse  # noqa: F401
except ImportError:  # pragma: no cover
    sys.path.insert(0, "/opt/trn_rl_repo")

import ml_dtypes

B, C, CO, N = 8, 128, 64, 4096
W = H = 64
NCORES = 8
IBLK = 512          # query columns per i-block
NJT = N // 128      # 32 j-tiles of 128 keys
NIB = N // IBLK     # 8 i-blocks
NPAIR = NJT // 2    # 16 j-tile pairs per i-block

# Schraudolph fast-exp constants (bf16 target, see module docstring)
A_SCH = 128.0 / np.log(2.0)
B_SCH = 8388608.0 + 16256.0 - 5.605

# exp engine assignment per pair slot: 'S' = ScalarE ACT, 'D' = DVE fast-exp.
PATTERN = ("S", "D") * (NPAIR // 2)

_CACHE = {}


def _split_multiwaits(nc):
    """Workaround for the pinned walrus: it accepts at most ONE semaphore wait
    per instruction (setupSyncWait: "Too many sync wait commands").  Hoist all
    but the last wait of any instruction onto single-wait NoOps inserted just
    before it in the same engine's stream — semantically identical (the engine
    blocks on each wait in turn before issuing the instruction)."""
    from concourse import mybir

    nsplit = 0
    for fn in nc.m.functions:
        for bb in fn.blocks:
            out = []
            for inst in bb.instructions:
                si = inst.sync_info
                if si is not None and si.on_wait is not None and len(si.on_wait) > 1:
                    waits = list(si.on_wait)
                    for i, w in enumerate(waits[:-1]):
                        out.append(mybir.InstNoOp(
                            name=f"{inst.name}-sw{i}",
                            engine=inst.engine,
                            sync_info=mybir.SyncInfo(on_wait=[w], on_update=[]),
                            bass_nofuse=True,
                        ))
                        nsplit += 1
                    si.on_wait = [waits[-1]]
                    inst.sync_info = si
                out.append(inst)
            bb.instructions = out
    return nsplit


def build_nc(debug=False, nib=NIB, pattern=PATTERN):
    from concourse import mybir
    import concourse.bass as bass
    import concourse.tile as tile

    f32 = mybir.dt.float32
    bf16 = mybir.dt.bfloat16
    Alu = mybir.AluOpType
    Act = mybir.ActivationFunctionType

    nc = bass.Bass()

    x_d = nc.dram_tensor("x", [C, N], f32, kind="ExternalInput")
    xb_d = nc.dram_tensor("xb", [C, N], bf16, kind="ExternalInput")  # host cast
    # packed bf16 weights:
    #   [wqT dup (128) | wkT dup (128) | wvT (64) | woT_aug (128, rows 0:65)]
    wpack_d = nc.dram_tensor("wpack", [C, 448], bf16, kind="ExternalInput")
    # packed f32 scalars: [bq | bk]
    bpack_d = nc.dram_tensor("bpack", [C, 2], f32, kind="ExternalInput")
    y_d = nc.dram_tensor("y", [C, N], f32, kind="ExternalOutput")

    with tile.TileContext(nc) as tc:
        with (
            tc.tile_pool(name="consts", bufs=1) as consts,
            tc.tile_pool(name="big", bufs=1) as big,
            tc.tile_pool(name="pts", bufs=3) as pts_pool,   # ScalarE exp out
            tc.tile_pool(name="ptd", bufs=3) as ptd_pool,   # DVE fast-exp out
            tc.tile_pool(name="epi", bufs=2) as epi,
        ):
            # ---- x chunk 0 DMA first (critical path), then packed weights ----
            x_sb = big.tile([C, N], f32)
            x_bf = big.tile([C, N], bf16)
            nc.sync.dma_start(x_bf[:, 0:512], xb_d[:, 0:512])

            wpack = consts.tile([C, 448], bf16)
            nc.gpsimd.dma_start(wpack, wpack_d[:, :])
            bpack = consts.tile([C, 2], f32)
            nc.gpsimd.dma_start(bpack, bpack_d[:, :])
            wqT = wpack[:, 0:128]
            wkT = wpack[:, 128:256]
            wvT = wpack[:, 256:320]
            woT = wpack[0:CO + 1, 320:448]   # [65, 128]: rows 0:64 w, row 64 gbo
            bq_s = bpack[:, 0:1]
            bk_s = bpack[:, 1:2]
            ones_bf = consts.tile([C, CO], bf16)
            nc.vector.memset(ones_bf, 1.0)

            # warm the natural_log_exp table set (~2.7us ACT_TABLE_LOAD) during
            # the DMA ramp; Ln first so the combined set (Ln+Exp+Identity) is
            # the one that gets loaded.
            warm = consts.tile([C, 1], f32)
            nc.vector.memset(warm, 1.0)
            nc.scalar.activation(warm, warm, Act.Ln)
            nc.scalar.activation(warm, warm, Act.Exp)

            # ---- x load + cast + projections, pipelined in 512-col chunks ----
            q_sb = big.tile([C, N], bf16)
            k_sb = big.tile([C, N], bf16)
            vT = big.tile([C, NJT * (CO + 1)], bf16)  # 32 x [128, 65] tiles
            vT3 = vT.rearrange("p (t e) -> p t e", e=CO + 1)
            nc.vector.memset(vT3[:, :, CO:CO + 1], 1.0)

            with tc.tile_pool(name="setup_ps", bufs=4, space="PSUM") as setup_ps:
                for t in range(N // 512):
                    sl = slice(t * 512, (t + 1) * 512)
                    if t > 0:  # chunk 0 DMA already issued above
                        nc.sync.dma_start(x_bf[:, sl], xb_d[:, sl])
                    # x f32 (residual add) loads independently, off the
                    # projection critical path
                    nc.sync.dma_start(x_sb[:, sl], x_d[:, sl])
                    ps_q = setup_ps.tile([C, 512], f32, tag="proj")
                    nc.tensor.matmul(ps_q, lhsT=wqT, rhs=x_bf[:, sl],
                                     start=True, stop=True)
                    nc.vector.tensor_scalar_add(q_sb[:, sl], ps_q, bq_s)
                    ps_k = setup_ps.tile([C, 512], f32, tag="proj")
                    nc.tensor.matmul(ps_k, lhsT=wkT, rhs=x_bf[:, sl],
                                     start=True, stop=True)
                    # k bias on ScalarE (idle until the first exp) so setup
                    # is not paced by DVE alone
                    nc.scalar.activation(k_sb[:, sl], ps_k, Act.Identity,
                                         bias=bk_s)
                    ps_v = setup_ps.tile([C, 256], f32, tag="vt")
                    for tt in range(4):
                        nt = t * 4 + tt
                        nc.tensor.matmul(
                            ps_v[:, tt * CO:(tt + 1) * CO],
                            lhsT=x_bf[:, nt * 128:(nt + 1) * 128],
                            rhs=wvT,
                            start=True, stop=True,
                        )
                    nc.vector.tensor_copy(
                        vT3[:, t * 4:(t + 1) * 4, 0:CO],
                        ps_v.rearrange("p (t e) -> p t e", e=CO),
                    )

            # ---- main loop over query blocks ----
            with (
                tc.tile_pool(name="qk_ps", bufs=3, space="PSUM") as qk_ps_pool,
                tc.tile_pool(name="pv_ps", bufs=1, space="PSUM") as pv_ps_pool,
                tc.tile_pool(name="epi_ps", bufs=1, space="PSUM") as epi_ps_pool,
            ):
                for ib in range(nib):
                    isl = slice(ib * IBLK, (ib + 1) * IBLK)
                    ps_u = pv_ps_pool.tile([CO + 1, IBLK], f32, tag="u")
                    # software pipeline: QK/exp run PIPE pairs ahead of PV so
                    # the PE never waits on an exp in flight.
                    PIPE = 2
                    stages = []  # (j0, rhs0, rhs1)
                    for pi in range(NPAIR + PIPE):
                        if pi < NPAIR:
                            j0 = 2 * pi
                            ps_qk = qk_ps_pool.tile([128, 1024], f32)
                            for idx in range(2):
                                jt = j0 + idx
                                half = jt % 2
                                hsl = slice(half * CO, half * CO + CO)
                                nc.tensor.matmul(
                                    ps_qk[:, idx * 512:(idx + 1) * 512],
                                    lhsT=k_sb[hsl, jt * 128:(jt + 1) * 128],
                                    rhs=q_sb[hsl, isl],
                                    start=True, stop=True,
                                )
                            if pattern[pi % len(pattern)] == "S":
                                pT = pts_pool.tile([128, 1024], bf16)
                                nc.scalar.activation(pT, ps_qk, Act.Exp)
                                stages.append((j0, pT[:, 0:512], pT[:, 512:1024]))
                            else:
                                sch = ptd_pool.tile([128, 1024], f32)
                                nc.vector.tensor_scalar(
                                    out=sch, in0=ps_qk,
                                    scalar1=float(A_SCH), scalar2=float(B_SCH),
                                    op0=Alu.mult, op1=Alu.add,
                                )
                                v_bf = sch[:, :].bitcast(bf16)
                                stages.append((j0, v_bf[:, 0:1024:2],
                                               v_bf[:, 1024:2048:2]))
                        if pi >= PIPE:
                            j0, rhs0, rhs1 = stages[pi - PIPE]
                            for idx, rhs in ((0, rhs0), (1, rhs1)):
                                jt = j0 + idx
                                nc.tensor.matmul(
                                    ps_u, lhsT=vT3[:, jt, :], rhs=rhs,
                                    start=(jt == 0), stop=(jt == NJT - 1),
                                )

                    # ---- epilogue ----
                    # ScalarE: unnormalized U -> SBUF (frees the U psum bank
                    # for the next block), then rden = Exp(-Ln(D)).
                    u_sb = epi.tile([CO, IBLK], bf16, tag="usb")
                    nc.scalar.activation(u_sb, ps_u[0:CO, :], Act.Identity)
                    lnd = epi.tile([CO + 1, IBLK], f32, tag="lnd")
                    nc.scalar.activation(lnd[CO:CO + 1, :], ps_u[CO:CO + 1, :],
                                         Act.Ln)
                    rden = epi.tile([CO + 1, IBLK], bf16, tag="rdn")
                    nc.scalar.activation(rden[CO:CO + 1, :], lnd[CO:CO + 1, :],
                                         Act.Exp, scale=-1.0)
                    # PE: K=1 ones-matmul broadcast of rden to partitions 0:64
                    ps_rdb = epi_ps_pool.tile([CO, IBLK], f32, tag="epi")
                    nc.tensor.matmul(ps_rdb, lhsT=ones_bf[CO:CO + 1, :],
                                     rhs=rden[CO:CO + 1, :],
                                     start=True, stop=True)
                    # DVE: normalize (ones row 64 of ub preset at first use)
                    ub = epi.tile([CO + 1, IBLK], bf16, tag="ub")
                    if ib < 2:
                        nc.vector.memset(ub[CO:CO + 1, :], 1.0)
                    nc.vector.tensor_tensor(out=ub[0:CO, :], in0=ps_rdb,
                                            in1=u_sb, op=Alu.mult)
                    # PE: output projection (o=65 contraction; row 64 = gbo)
                    ps_oc = epi_ps_pool.tile([C, IBLK], f32, tag="epi")
                    nc.tensor.matmul(ps_oc, lhsT=woT, rhs=ub[:, :],
                                     start=True, stop=True)
                    # DVE: residual add, DMA out
                    y2 = epi.tile([C, IBLK], f32, tag="y2")
                    nc.vector.tensor_tensor(out=y2, in0=ps_oc,
                                            in1=x_sb[:, isl], op=Alu.add)
                    nc.sync.dma_start(y_d[:, isl], y2)

    _split_multiwaits(nc)
    return nc


def host_prep(inputs):
    """Fold scales/transposes on the host; returns the 8 per-core input maps."""
    x = np.ascontiguousarray(np.asarray(inputs["x"], dtype=np.float32))
    wq = np.asarray(inputs["wq"], dtype=np.float32)
    bq = np.asarray(inputs["bq"], dtype=np.float32)
    wk = np.asarray(inputs["wk"], dtype=np.float32)
    bk = np.asarray(inputs["bk"], dtype=np.float32)
    wv = np.asarray(inputs["wv"], dtype=np.float32)
    bv = np.asarray(inputs["bv"], dtype=np.float32)
    wo = np.asarray(inputs["wo"], dtype=np.float32)
    bo = np.asarray(inputs["bo"], dtype=np.float32)
    gamma = float(np.asarray(inputs["gamma"]).reshape(-1)[0])

    s = 1.0 / np.sqrt(np.float32(C))
    bf = ml_dtypes.bfloat16
    wqTs = wq.T * s                                                    # [128,64]
    wqT = np.concatenate([wqTs, wqTs], axis=1)                         # [128,128]
    wkT = np.concatenate([wk.T, wk.T], axis=1)                         # [128,128]
    wvT = wv.T                                                         # [128,64]
    gbo = gamma * (wo @ bv + bo)                                       # [128]
    woT_aug = np.zeros((C, C), np.float32)
    woT_aug[:CO, :] = gamma * wo.T                                     # rows 0:64
    woT_aug[CO, :] = gbo                                               # row 64
    wpack = np.concatenate([wqT, wkT, wvT, woT_aug], axis=1).astype(bf)
    bq_s = np.concatenate([bq * s, bq * s])
    bk_s = np.concatenate([bk, bk])
    bpack = np.stack([bq_s, bk_s], axis=1).astype(np.float32)          # [128,2]

    xb = x.reshape(B, C, N)
    in_maps = []
    for b in range(B):
        in_maps.append({
            "x": np.ascontiguousarray(xb[b]),
            "xb": np.ascontiguousarray(xb[b].astype(bf)),
            "wpack": wpack, "bpack": bpack,
        })
    return in_maps


def run(inputs, trace=False, **kw):
    from concourse.bass_utils import run_bass_kernel_spmd

    if "nc" not in _CACHE:
        _CACHE["nc"] = build_nc()
    nc = _CACHE["nc"]
    in_maps = host_prep(inputs)
    try:
        res = run_bass_kernel_spmd(nc, in_maps, core_ids=list(range(NCORES)),
                                   trace=trace, **kw)
    except Exception:
        # transient device wedge (e.g. NRT_EXEC_UNIT_UNRECOVERABLE from an
        # earlier crashed process) -- retry once
        res = run_bass_kernel_spmd(nc, in_maps, core_ids=list(range(NCORES)),
                                   trace=trace, **kw)
    y = np.stack([np.asarray(res.results[b]["y"]) for b in range(B)])
    y = y.reshape(B, C, W, H).astype(np.float32)
    return y, res


def kernel(**inputs) -> np.ndarray:
    y, _ = run(inputs)
    return y
